# revision 1
# baseline (speedup 1.0000x reference)
"""CPRRouter (MoE cosine-sim routing) Trainium2 kernel.

Full inputs: hidden_states [16384, 2048] f32, proto [64, 2048] f32.
Returns (topk_weights [16384, 8] f32, selected_experts [16384, 8] int32),
matching jax: softmax(cos_sim(l2norm(h), l2norm(proto))) -> top_k(8).

Sharding: data-parallel over tokens across 8 NeuronCores (2048 tokens/core),
proto replicated.

Per-core pipeline (16 token-tiles of 128 tokens):
  - DMA h tile [128, 2048] (natural layout)
  - ACT: square + accum_out -> ssq per token; sqrt; DVE reciprocal -> 1/||h||
  - PE: 16x fp32 transpose [128,128] -> hT chunks (via PSUM, DVE copy to SBUF)
  - PE: 16x fp32 matmul (lhsT=hT chunk, rhs=pnT chunk) accumulating
        D[t, e] = h . l2norm(proto)  in PSUM [128, 64]
  - ACT: exp(D * 1/||h||) with accum_out -> sumexp (softmax denominator)
  - DVE: max8/max_index on raw dots (order == softmax order), then
        w8 = exp(top8 * 1/||h||) * (1/sumexp)
"""
import sys

sys.path.insert(0, "/opt/trn_rl_repo")

import numpy as np

N_CORES = 8
T_FULL, H, E = 16384, 2048, 64
T_CORE = T_FULL // N_CORES          # 2048 tokens per core
N_TILES = T_CORE // 128             # 16 token tiles
KC = H // 128                       # 16 contraction chunks

_nc_cache = None


def _build():
    global _nc_cache
    if _nc_cache is not None:
        return _nc_cache

    import concourse.bass as bass  # noqa: F401
    import concourse.tile as tile
    from concourse import bacc, mybir
    from concourse.masks import make_identity

    f32 = mybir.dt.float32
    nc = bacc.Bacc("TRN2", target_bir_lowering=False, debug=False,
                   num_devices=N_CORES)
    hs = nc.dram_tensor("hidden_states", [T_CORE, H], f32,
                        kind="ExternalInput").ap()
    proto = nc.dram_tensor("proto", [E, H], f32, kind="ExternalInput").ap()
    w_out = nc.dram_tensor("w8", [T_CORE, 8], f32, kind="ExternalOutput").ap()
    i_out = nc.dram_tensor("i8", [T_CORE, 8], mybir.dt.uint32,
                           kind="ExternalOutput").ap()

    with tile.TileContext(nc) as tc:
        with (
            tc.tile_pool(name="persist", bufs=1) as persist,
            tc.tile_pool(name="hload", bufs=3) as hload,
            tc.tile_pool(name="sq", bufs=1) as sqp,
            tc.tile_pool(name="xt", bufs=2) as xtp,
            tc.tile_pool(name="small", bufs=2) as small,
            tc.tile_pool(name="tp", bufs=4, space="PSUM") as tp,
            tc.tile_pool(name="dp", bufs=2, space="PSUM") as dp,
        ):
            ident = persist.tile([128, 128], f32)
            make_identity(nc, ident)

            # ---- proto: load, l2-normalize, transpose -> pnT chunks ----
            p_sb = persist.tile([E, H], f32)
            nc.sync.dma_start(p_sb, proto)
            p_sq = persist.tile([E, H], f32)
            p_ssq = persist.tile([E, 1], f32)
            nc.scalar.activation(p_sq, p_sb, mybir.ActivationFunctionType.Square,
                                 accum_out=p_ssq)
            p_norm = persist.tile([E, 1], f32)
            nc.scalar.sqrt(p_norm, p_ssq)
            p_rnorm = persist.tile([E, 1], f32)
            nc.vector.reciprocal(p_rnorm, p_norm)
            pn_sb = persist.tile([E, H], f32)
            nc.vector.tensor_scalar_mul(pn_sb, p_sb, p_rnorm)
            pnT = persist.tile([128, KC * E], f32)
            for k in range(KC):
                pnT_ps = tp.tile([128, 128], f32, tag="tp")
                nc.tensor.transpose(pnT_ps[:, :E], pn_sb[:, k * 128:(k + 1) * 128],
                                    ident[:E, :E])
                nc.vector.tensor_copy(pnT[:, k * E:(k + 1) * E], pnT_ps[:, :E])

            # ---- token tiles ----
            for i in range(N_TILES):
                r0 = i * 128
                h_nat = hload.tile([128, H], f32)
                nc.sync.dma_start(h_nat, hs[r0:r0 + 128, :])

                x_sq = sqp.tile([128, H], f32)
                ssq = small.tile([128, 1], f32)
                nc.scalar.activation(x_sq, h_nat,
                                     mybir.ActivationFunctionType.Square,
                                     accum_out=ssq)
                norm = small.tile([128, 1], f32)
                nc.scalar.sqrt(norm, ssq)
                rnorm = small.tile([128, 1], f32)
                nc.vector.reciprocal(rnorm, norm)

                xT = xtp.tile([128, KC * 128], f32)
                for k in range(KC):
                    xT_ps = tp.tile([128, 128], f32, tag="tp")
                    nc.tensor.transpose(xT_ps, h_nat[:, k * 128:(k + 1) * 128],
                                        ident)
                    nc.vector.tensor_copy(xT[:, k * 128:(k + 1) * 128], xT_ps)

                d_ps = dp.tile([128, E], f32)
                for k in range(KC):
                    nc.tensor.matmul(d_ps, xT[:, k * 128:(k + 1) * 128],
                                     pnT[:, k * E:(k + 1) * E],
                                     start=(k == 0), stop=(k == KC - 1))
                d_sb = small.tile([128, E], f32)
                nc.vector.tensor_copy(d_sb, d_ps)

                e_sb = small.tile([128, E], f32)
                sumexp = small.tile([128, 1], f32)
                nc.scalar.activation(e_sb, d_ps, mybir.ActivationFunctionType.Exp,
                                     scale=rnorm, accum_out=sumexp)
                rsum = small.tile([128, 1], f32)
                nc.vector.reciprocal(rsum, sumexp)

                top_d = small.tile([128, 8], f32)
                nc.vector.max(out=top_d, in_=d_sb)
                idx = small.tile([128, 8], mybir.dt.uint32)
                nc.vector.max_index(out=idx, in_max=top_d, in_values=d_sb)
                top_e = small.tile([128, 8], f32)
                nc.scalar.activation(top_e, top_d,
                                     mybir.ActivationFunctionType.Exp,
                                     scale=rnorm)
                w8 = small.tile([128, 8], f32)
                nc.vector.tensor_scalar_mul(w8, top_e, rsum)
                nc.sync.dma_start(w_out[r0:r0 + 128, :], w8)
                nc.sync.dma_start(i_out[r0:r0 + 128, :], idx)

    nc.compile()
    _nc_cache = nc
    return nc


def _run(hidden_states, proto, trace=False, **trace_kwargs):
    from concourse.bass_utils import run_bass_kernel_spmd

    nc = _build()
    hidden_states = np.ascontiguousarray(hidden_states, dtype=np.float32)
    proto = np.ascontiguousarray(proto, dtype=np.float32)
    in_maps = [
        {"hidden_states": hidden_states[c * T_CORE:(c + 1) * T_CORE],
         "proto": proto}
        for c in range(N_CORES)
    ]
    res = run_bass_kernel_spmd(nc, in_maps, list(range(N_CORES)), trace=trace,
                               **trace_kwargs)
    w = np.concatenate([r["w8"] for r in res.results], axis=0)
    idx = np.concatenate([r["i8"] for r in res.results], axis=0)
    return (w.astype(np.float32), idx.astype(np.int32)), res


def kernel(hidden_states, proto):
    out, _ = _run(hidden_states, proto)
    return out
